# revision 7
# baseline (speedup 1.0000x reference)
"""Trainium2 Bass kernel for a single causal attention head (with the
faithful source bug: q = x @ W_key, W_query unused).

Full-input contract: kernel(x, W_key, W_query, W_value) -> [8, 2048, 128].
Sharding: data-parallel over batch B=8 across 8 NeuronCores (1 batch/core).

Per-core math (T=2048, C=1024, H=128):
    K = x @ W_key            (V = x @ W_value)
    S = K @ K.T * H**-0.5    (symmetric since q == k)
    out = softmax(causal(S)) @ V

v2 schedule (PE-bound end to end; streams ~28.6us of fp16 columns):
  - All input DMA triggers issue first: wk/wv split so the first
    LDWEIGHTS gates on a 64-descriptor transfer, xt0 split in half so
    the first matmul gates on 32 descriptors. Triggers spread across
    the sync and gpsimd queues; the scalar queue holds only the exp
    table warm so no trigger ever queues behind ACT_TABLE_LOAD.
  - Projections run tile-major (K then V per c-tile) chasing the DMA.
    On the last c-tile the K chunks are emitted first with the
    PSUM->SBUF fp16 casts interleaved on scalar/vector, and the V
    chunks keep the PE busy while those casts drain.
  - Scores row j: S tile [keys j (part), queries b (free)] = KT_j^T @
    KT_b, upper triangle only (S symmetric). exp on ScalarE in
    1024-col chunks (one ACT instr spans 2 PSUM banks) without max
    subtraction; causal mask is a post-exp multiply on the diag tile
    (gpsimd, so the DVE stays on normalize duty).
  - AV columns accumulate in PSUM under a WIN=3 sliding window (one
    open accumulation group per 2KB bank); softmax denominators ride
    as a ones column appended to V. AV matmuls are interleaved
    between score-chunk matmuls so their LDWEIGHTS (~97ns) hide under
    512-col matmuls (~216ns).
  - PSUM layout (16KB/partition exactly): sps tag 2x4KB (K proj accum
    in A, score chunks in B), av tag 3x2KB (V proj ch0-2 in A, AV
    window in B), vtr tag 1x2KB (V proj ch3 in A, V-transpose in B).
"""

import numpy as np

import concourse.bass as bass
import concourse.mybir as mybir
import concourse.tile as tile
from concourse import bacc, bass_utils
from concourse.masks import make_identity, make_upper_triangular


P = 128
T = 2048
C = 1024
H = 128
NT = T // P  # 16 seq tiles
NC = C // P  # 8 contraction tiles
NCORES = 8
SCALE = float(H) ** -0.5
F32 = mybir.dt.float32
FP16 = mybir.dt.float16
EXP = mybir.ActivationFunctionType.Exp
CHW = 512
CHN = T // CHW  # 4 chunks
WIN = 3
ECHUNK = 1024  # exp chunk width (spans 2 PSUM banks)


def build_module():
    nc = bacc.Bacc(
        "TRN2", target_bir_lowering=False, debug=False, num_devices=NCORES
    )
    xT_d = nc.dram_tensor("xT", [C, T], FP16, kind="ExternalInput").ap()
    wk_d = nc.dram_tensor("WK", [P, NC, H], FP16, kind="ExternalInput").ap()
    wv_d = nc.dram_tensor("WV", [P, NC, H], FP16, kind="ExternalInput").ap()
    y_d = nc.dram_tensor("y", [T, H], F32, kind="ExternalOutput").ap()

    # offsets of score row-block j inside e_all (block j holds queries
    # b in [j*128, 2048) -> width (NT-j)*128)
    offs = []
    off = 0
    for j in range(NT):
        offs.append(off)
        off += (NT - j) * P
    e_width = off  # 136 * 128 = 17408

    with tile.TileContext(nc) as tc:
        with (
            tc.tile_pool(name="const", bufs=1) as const,
            tc.tile_pool(name="xt", bufs=8) as xt_pool,
            tc.tile_pool(name="kv", bufs=1) as kv,
            tc.tile_pool(name="e", bufs=1) as e_pool,
            tc.tile_pool(name="outp", bufs=4) as outp,
            tc.tile_pool(name="ps", bufs=1, space="PSUM") as ps,
        ):
            # ---- input DMA triggers first, nothing ahead of them ----
            wk_sb = const.tile([P, NC, H], FP16)
            wv_sb = const.tile([P, NC, H], FP16)
            xt = [
                xt_pool.tile([P, T], FP16, tag="xt", name=f"xt{c}")
                for c in range(NC)
            ]
            # sync queue: wk, wv, odd x tiles
            nc.sync.dma_start(wk_sb[:], wk_d[:])
            nc.sync.dma_start(wv_sb[:], wv_d[:])
            # gpsimd queue: xt0 split in half (first matmul gates on
            # 32 descriptors), even x tiles
            nc.gpsimd.dma_start(xt[0][:, 0:CHW], xT_d[0:P, 0:CHW])
            nc.sync.dma_start(xt[1][:], xT_d[P : 2 * P, :])
            nc.gpsimd.dma_start(xt[0][:, CHW:T], xT_d[0:P, CHW:T])
            nc.sync.dma_start(xt[3][:], xT_d[3 * P : 4 * P, :])
            nc.gpsimd.dma_start(xt[2][:], xT_d[2 * P : 3 * P, :])
            nc.sync.dma_start(xt[5][:], xT_d[5 * P : 6 * P, :])
            nc.gpsimd.dma_start(xt[4][:], xT_d[4 * P : 5 * P, :])
            nc.sync.dma_start(xt[7][:], xT_d[7 * P : 8 * P, :])
            nc.gpsimd.dma_start(xt[6][:], xT_d[6 * P : 7 * P, :])

            # scalar queue: pre-warm the ACT exp table immediately
            warm = const.tile([P, 1], F32)
            nc.vector.memset(warm[:], 0.0)
            nc.scalar.activation(warm[:], warm[:], EXP)

            # constants: gpsimd builds them after its triggers; fp16
            # casts also on gpsimd (vector stays free for A-end casts)
            ident_f = const.tile([P, P], F32)
            make_identity(nc, ident_f)
            dmask_f = const.tile([P, P], F32)
            make_upper_triangular(nc, dmask_f, val=1.0, diag=True)
            ident = const.tile([P, P], FP16)
            nc.gpsimd.tensor_copy(ident[:], ident_f[:])
            dmask = const.tile([P, P], FP16)
            nc.gpsimd.tensor_copy(dmask[:], dmask_f[:])

            kt_r = kv.tile([P, T], FP16)  # K^T [h, t]
            vt_sb = kv.tile([P, T], FP16)  # V^T [h, t]
            # per key-tile j: [v (128) | ones (1)]
            vaug = kv.tile([P, NT, P + 1], FP16)
            nc.vector.memset(vaug[:, :, P : P + 1], 1.0)
            e_all = e_pool.tile([P, e_width], FP16)

            # ---- PSUM tiles: 16KB/partition budget, exactly ----
            # sps: 2 x [128,1024] f32 (4KB, 2 banks each)
            # av:  3 x [128,512]  f32 (2KB each)
            # vtr: 1 x [128,512]  f32 (2KB)
            kt_ps = [
                ps.tile([P, ECHUNK], F32, tag="sps", name=f"ktps{h}")
                for h in range(2)
            ]
            vt_ps = [
                ps.tile([P, CHW], F32, tag="av", name=f"vtps{ch}")
                for ch in range(3)
            ] + [ps.tile([P, CHW], F32, tag="vtr", name="vtps3")]

            def kt_slice(ch):
                return kt_ps[ch // 2][:, (ch % 2) * CHW : (ch % 2 + 1) * CHW]

            # ---- A: projections, tile-major, chasing the DMA ----
            for c in range(NC):
                last = c == NC - 1
                for ch in range(CHN):
                    rhs = xt[c][:, ch * CHW : (ch + 1) * CHW]
                    nc.tensor.matmul(
                        kt_slice(ch),
                        wk_sb[:, c, :],
                        rhs,
                        start=(c == 0),
                        stop=last,
                    )
                    if last:
                        # casts interleave on scalar/vector while the
                        # remaining proj matmuls keep the PE busy
                        sl = slice(ch * CHW, (ch + 1) * CHW)
                        eng = nc.scalar if ch % 2 == 0 else nc.vector
                        if ch % 2 == 0:
                            eng.copy(kt_r[:, sl], kt_slice(ch))
                        else:
                            eng.tensor_copy(kt_r[:, sl], kt_slice(ch))
                for ch in range(CHN):
                    rhs = xt[c][:, ch * CHW : (ch + 1) * CHW]
                    nc.tensor.matmul(
                        vt_ps[ch][:],
                        wv_sb[:, c, :],
                        rhs,
                        start=(c == 0),
                        stop=last,
                    )
            # vt casts on vector; ch3 early to free the vtr slot for
            # the first V-transpose; ch1/ch2 deferred until after
            # transpose_v(0) so the vaug0 copy isn't queued behind them
            for ch in (0, 3):
                sl = slice(ch * CHW, (ch + 1) * CHW)
                nc.vector.tensor_copy(vt_sb[:, sl], vt_ps[ch][:])

            # ---- B: scores row j / exp / V-transpose / AV / out ----
            def scores_row(j):
                """Emit score matmul+exp chunks for row j as thunk pairs
                so AV matmuls can interleave between them."""
                b0 = j * P
                width = T - b0
                thunks = []
                pos = 0
                while pos < width:
                    w = min(ECHUNK, width - pos)
                    cpos, cw = pos, w

                    def chunk(cpos=cpos, cw=cw, j=j, b0=b0):
                        s_ps = ps.tile(
                            [P, ECHUNK], F32, tag="sps", name=f"sps{j}_{cpos}"
                        )
                        p2 = 0
                        while p2 < cw:
                            w2 = min(CHW, cw - p2)
                            nc.tensor.matmul(
                                s_ps[:, p2 : p2 + w2],
                                kt_r[:, b0 : b0 + P],
                                kt_r[:, b0 + cpos + p2 : b0 + cpos + p2 + w2],
                                start=True,
                                stop=True,
                            )
                            p2 += w2
                        nc.scalar.activation(
                            e_all[:, offs[j] + cpos : offs[j] + cpos + cw],
                            s_ps[:, :cw],
                            EXP,
                            scale=SCALE,
                        )
                        if cpos == 0:
                            # causal mask on the diag tile only (gpsimd)
                            nc.gpsimd.tensor_mul(
                                e_all[:, offs[j] : offs[j] + P],
                                e_all[:, offs[j] : offs[j] + P],
                                dmask[:],
                            )

                    thunks.append(chunk)
                    pos += w
                return thunks

            def transpose_v(j):
                vtr = ps.tile([P, CHW], FP16, tag="vtr", name=f"vtr{j}")
                nc.tensor.transpose(
                    vtr[:, :P], vt_sb[:, j * P : (j + 1) * P], ident[:]
                )
                nc.vector.tensor_copy(vaug[:, j, 0:P], vtr[:, :P])

            av_banks = {}

            def av_update(j, i, start, stop):
                eji = e_all[
                    :, offs[j] + (i - j) * P : offs[j] + (i - j + 1) * P
                ]
                nc.tensor.matmul(
                    av_banks[i][:, : P + 1],
                    eji,
                    vaug[:, j, :],
                    start=start,
                    stop=stop,
                )

            def normalize_out(i):
                av = av_banks[i][:, : P + 1]
                recip = outp.tile([P, 1], F32, tag="recip", name=f"rcp{i}")
                nc.vector.reciprocal(recip[:], av[:, P : P + 1])
                o_sb = outp.tile([P, H], F32, tag="osb", name=f"osb{i}")
                nc.vector.tensor_scalar_mul(o_sb[:], av[:, 0:P], recip[:])
                nc.sync.dma_start(y_d[i * P : (i + 1) * P, :], o_sb[:])

            # row 0 + first transpose before the round loop
            for th in scores_row(0):
                th()
            transpose_v(0)
            for ch in (1, 2):
                sl = slice(ch * CHW, (ch + 1) * CHW)
                nc.vector.tensor_copy(vt_sb[:, sl], vt_ps[ch][:])

            for j in range(NT):
                # build this round's AV thunk list: window updates for
                # cols > j first, catch-up for the newly activated
                # column, the diag update (j, j) last so the gpsimd
                # mask has landed by the time the PE reaches it
                avs = []
                if j == 0:
                    for i in range(min(WIN, NT)):
                        av_banks[i] = ps.tile(
                            [P, CHW], F32, tag="av", name=f"avb{i}"
                        )
                    # (0,0) first: its bank slot (ex vtps0) frees
                    # earliest; cols 1/2 wait on the vt1/vt2 casts
                    avs.append(lambda: av_update(0, 0, start=True, stop=True))
                    for i in range(1, min(WIN, NT)):
                        avs.append(
                            lambda i=i: av_update(0, i, start=True, stop=False)
                        )
                else:
                    act = j + WIN - 1
                    hi = min(j + WIN - 1, NT)
                    for i in range(j + 1, hi):
                        avs.append(
                            lambda i=i, j=j: av_update(
                                j, i, start=False, stop=False
                            )
                        )
                    if act < NT:
                        av_banks[act] = ps.tile(
                            [P, CHW], F32, tag="av", name=f"avb{act}"
                        )
                        for jc in range(j + 1):
                            avs.append(
                                lambda jc=jc, act=act: av_update(
                                    jc, act, start=(jc == 0), stop=False
                                )
                            )
                    avs.append(
                        lambda j=j: av_update(j, j, start=False, stop=(True))
                    )

                # interleave: score chunks of row j+1 between AV bursts
                chunks = scores_row(j + 1) if j + 1 < NT else []
                n_ch = len(chunks)
                emitted = 0
                for ci, ch_th in enumerate(chunks):
                    ch_th()
                    if ci == 0 and j + 1 < NT:
                        transpose_v(j + 1)
                    # spread AVs evenly across the chunks
                    take = (len(avs) * (ci + 1)) // n_ch - emitted
                    for th in avs[emitted : emitted + take]:
                        th()
                    emitted += take
                for th in avs[emitted:]:
                    th()
                normalize_out(j)

    nc.compile()
    return nc


_NC_CACHE = None


def _get_module():
    global _NC_CACHE
    if _NC_CACHE is None:
        _NC_CACHE = build_module()
    return _NC_CACHE


def run(in_maps, trace=False, **kw):
    nc = _get_module()
    return bass_utils.run_bass_kernel_spmd(
        nc, in_maps, core_ids=list(range(NCORES)), trace=trace, **kw
    )


def make_in_maps(x, W_key, W_value):
    x = np.asarray(x, dtype=np.float32).astype(np.float16)
    xT = np.ascontiguousarray(x.transpose(0, 2, 1))
    wk = np.asarray(W_key, np.float32).astype(np.float16)
    wk = np.ascontiguousarray(wk.reshape(NC, P, H).transpose(1, 0, 2))
    wv = np.asarray(W_value, np.float32).astype(np.float16)
    wv = np.ascontiguousarray(wv.reshape(NC, P, H).transpose(1, 0, 2))
    return [{"xT": xT[b], "WK": wk, "WV": wv} for b in range(NCORES)]


def kernel(x, W_key, W_query, W_value):
    # W_query intentionally unused: the reference applies W_key for q too.
    del W_query
    res = run(make_in_maps(x, W_key, W_value), trace=False)
    return np.stack([res.results[b]["y"] for b in range(NCORES)], axis=0)


# revision 9
# speedup vs baseline: 1.6776x; 1.6776x over previous
"""Trainium2 Bass kernel for a single causal attention head (with the
faithful source bug: q = x @ W_key, W_query unused).

Full-input contract: kernel(x, W_key, W_query, W_value) -> [8, 2048, 128].
Sharding: data-parallel over batch B=8 across 8 NeuronCores (1 batch/core).

Per-core math (T=2048, C=1024, H=128):
    K = x @ W_key            (V = x @ W_value)
    S = K @ K.T * H**-0.5    (symmetric since q == k)
    out = softmax(causal(S)) @ V

Schedule:
  - All input DMA triggers issue before anything else, on hardware-DGE
    queues only (sync/scalar/vector — gpsimd dma_start is software-DGE
    at ~750ns/trigger and must not carry input DMAs). wk/wv are split
    tensors so the first LDWEIGHTS gates on a 64-descriptor transfer;
    xt0 is DMA'd in two halves so the first matmul gates on 32
    descriptors. The ACT exp-table warm runs after the scalar queue's
    triggers; constants (identity/diag mask) build on gpsimd which has
    nothing else to do.
  - Projections run tile-major (K chunks then V chunks per c-tile),
    chasing the DMA. On the last c-tile the K chunks are emitted first
    with the PSUM->SBUF fp16 casts interleaved on scalar/vector, and
    the V chunks keep the PE busy while those casts drain. vt casts
    ch0/ch3 precede transpose_v(0) (vaug0 ASAP), ch1/ch2 follow it.
  - K kept transposed (KT [h, t], fp16). Score tile (j-rows, b-cols) =
    KT_j.T @ KT_b -> [keys j (part), queries b (free)] which is exactly
    the AV lhsT layout. Only the upper triangle is computed (S
    symmetric); causal mask is a post-exp multiply on diag tiles only.
  - exp without max-subtraction (scores bounded; fp16 E in range) in
    512-col chunks on ScalarE; softmax denominators ride the AV matmul
    as a ones-column appended to V (rhs = [v | 1], 129 cols).
  - AV columns accumulate in PSUM under a WIN=3 sliding window (one
    open accumulation group per 2KB bank); a late-activated column
    catches up earlier rows from e_all. One-round software pipeline:
    scores row j+1 issues before row j's AV updates so the in-order PE
    never waits on ScalarE's exp.
"""

import numpy as np

import concourse.bass as bass
import concourse.mybir as mybir
import concourse.tile as tile
from concourse import bacc, bass_utils
from concourse.masks import make_identity, make_upper_triangular


P = 128
T = 2048
C = 1024
H = 128
NT = T // P  # 16 seq tiles
NC = C // P  # 8 contraction tiles
NCORES = 8
SCALE = float(H) ** -0.5
F32 = mybir.dt.float32
FP16 = mybir.dt.float16
EXP = mybir.ActivationFunctionType.Exp
CHW = 512
CHN = T // CHW  # 4 chunks


def build_module():
    nc = bacc.Bacc(
        "TRN2", target_bir_lowering=False, debug=False, num_devices=NCORES
    )
    xT_d = nc.dram_tensor("xT", [C, T], FP16, kind="ExternalInput").ap()
    wk_d = nc.dram_tensor("WK", [P, NC, H], FP16, kind="ExternalInput").ap()
    wv_d = nc.dram_tensor("WV", [P, NC, H], FP16, kind="ExternalInput").ap()
    y_d = nc.dram_tensor("y", [T, H], F32, kind="ExternalOutput").ap()

    # offsets of score row-block j inside e_all (block j holds queries
    # b in [j*128, 2048) -> width (NT-j)*128)
    offs = []
    off = 0
    for j in range(NT):
        offs.append(off)
        off += (NT - j) * P
    e_width = off  # 136 * 128 = 17408

    with tile.TileContext(nc) as tc:
        with (
            tc.tile_pool(name="const", bufs=1) as const,
            tc.tile_pool(name="xt", bufs=8) as xt_pool,
            tc.tile_pool(name="kv", bufs=1) as kv,
            tc.tile_pool(name="e", bufs=1) as e_pool,
            tc.tile_pool(name="outp", bufs=4) as outp,
            tc.tile_pool(name="ps", bufs=8, space="PSUM") as ps,
        ):
            # ---- input DMA triggers first, nothing ahead of them ----
            wk_sb = const.tile([P, NC, H], FP16)
            wv_sb = const.tile([P, NC, H], FP16)
            xt = [
                xt_pool.tile([P, T], FP16, tag="xt", name=f"xt{c}")
                for c in range(NC)
            ]
            nc.sync.dma_start(wk_sb[:], wk_d[:])
            nc.scalar.dma_start(xt[0][:, 0:CHW], xT_d[0:P, 0:CHW])
            nc.sync.dma_start(wv_sb[:], wv_d[:])
            nc.scalar.dma_start(xt[0][:, CHW:T], xT_d[0:P, CHW:T])
            nc.sync.dma_start(xt[1][:], xT_d[P : 2 * P, :])
            nc.scalar.dma_start(xt[2][:], xT_d[2 * P : 3 * P, :])
            nc.sync.dma_start(xt[3][:], xT_d[3 * P : 4 * P, :])
            nc.scalar.dma_start(xt[4][:], xT_d[4 * P : 5 * P, :])
            nc.sync.dma_start(xt[5][:], xT_d[5 * P : 6 * P, :])
            nc.scalar.dma_start(xt[6][:], xT_d[6 * P : 7 * P, :])
            nc.sync.dma_start(xt[7][:], xT_d[7 * P : 8 * P, :])

            # pre-warm the ACT exp table (scalar queue, after triggers)
            warm = const.tile([P, 1], F32)
            nc.vector.memset(warm[:], 0.0)
            nc.scalar.activation(warm[:], warm[:], EXP)

            # constants on gpsimd (free of triggers); fp16 casts on
            # gpsimd too so the vector queue stays clear for A-end
            ident_f = const.tile([P, P], F32)
            make_identity(nc, ident_f)
            dmask_f = const.tile([P, P], F32)
            make_upper_triangular(nc, dmask_f, val=1.0, diag=True)
            ident = const.tile([P, P], FP16)
            nc.gpsimd.tensor_copy(ident[:], ident_f[:])
            dmask = const.tile([P, P], FP16)
            nc.gpsimd.tensor_copy(dmask[:], dmask_f[:])

            kt_r = kv.tile([P, T], FP16)  # K^T [h, t]
            vt_sb = kv.tile([P, T], FP16)  # V^T [h, t]
            # per key-tile j: [v (128) | ones (1)]
            vaug = kv.tile([P, NT, P + 1], FP16)
            nc.vector.memset(vaug[:, :, P : P + 1], 1.0)
            e_all = e_pool.tile([P, e_width], FP16)

            # ---- A: projections, tile-major, chasing the DMA ----
            kt_ps = [
                ps.tile([P, CHW], F32, tag="ps", name=f"ktps{ch}")
                for ch in range(CHN)
            ]
            vt_ps = [
                ps.tile([P, CHW], F32, tag="ps", name=f"vtps{ch}")
                for ch in range(CHN)
            ]
            for c in range(NC):
                last = c == NC - 1
                for ch in range(CHN):
                    rhs = xt[c][:, ch * CHW : (ch + 1) * CHW]
                    nc.tensor.matmul(
                        kt_ps[ch][:],
                        wk_sb[:, c, :],
                        rhs,
                        start=(c == 0),
                        stop=last,
                    )
                    if last:
                        # kt casts interleave on scalar/vector while
                        # the V proj matmuls keep the PE busy
                        sl = slice(ch * CHW, (ch + 1) * CHW)
                        if ch % 2 == 0:
                            nc.scalar.copy(kt_r[:, sl], kt_ps[ch][:])
                        else:
                            nc.vector.tensor_copy(kt_r[:, sl], kt_ps[ch][:])
                for ch in range(CHN):
                    rhs = xt[c][:, ch * CHW : (ch + 1) * CHW]
                    nc.tensor.matmul(
                        vt_ps[ch][:],
                        wv_sb[:, c, :],
                        rhs,
                        start=(c == 0),
                        stop=last,
                    )
            # vt casts: ch0/ch3 first (transpose_v(0) needs vt chunk 0
            # and the vtr psum slot is recycled from vt_ps bank ring),
            # ch1/ch2 after transpose_v(0) so vaug0 isn't queued late
            for ch in (0, 3):
                sl = slice(ch * CHW, (ch + 1) * CHW)
                nc.vector.tensor_copy(vt_sb[:, sl], vt_ps[ch][:])

            # ---- B: scores row j / exp / V-transpose j / AV col ----
            NAV = P + 1  # v | ones

            def scores_row(j):
                b0 = j * P
                width = T - b0
                pos = 0
                while pos < width:
                    w = min(CHW, width - pos)
                    s_ps = ps.tile([P, CHW], F32, tag="ps", name=f"sps{j}_{pos}")
                    nc.tensor.matmul(
                        s_ps[:, :w],
                        kt_r[:, b0 : b0 + P],
                        kt_r[:, b0 + pos : b0 + pos + w],
                        start=True,
                        stop=True,
                    )
                    nc.scalar.activation(
                        e_all[:, offs[j] + pos : offs[j] + pos + w],
                        s_ps[:, :w],
                        EXP,
                        scale=SCALE,
                    )
                    pos += w
                # causal mask only needed on the diagonal tile
                nc.vector.tensor_mul(
                    e_all[:, offs[j] : offs[j] + P],
                    e_all[:, offs[j] : offs[j] + P],
                    dmask[:],
                )

            def transpose_v(j):
                vtr = ps.tile([P, CHW], FP16, tag="ps", name=f"vtr{j}")
                nc.tensor.transpose(
                    vtr[:, :P], vt_sb[:, j * P : (j + 1) * P], ident[:]
                )
                nc.vector.tensor_copy(vaug[:, j, 0:P], vtr[:, :P])

            # AV columns accumulate in PSUM, at most one open
            # accumulation group per bank. A sliding window of WIN
            # concurrent columns: column i activates at round
            # max(0, i - WIN + 1), catches up rows 0..r-1 from e_all,
            # then takes one update per subsequent round.
            WIN = 3
            av_banks = {}

            def av_update(j, i, start, stop):
                eji = e_all[
                    :, offs[j] + (i - j) * P : offs[j] + (i - j + 1) * P
                ]
                nc.tensor.matmul(
                    av_banks[i][:, :NAV],
                    eji,
                    vaug[:, j, :],
                    start=start,
                    stop=stop,
                )

            def normalize_out(i):
                av = av_banks[i][:, :NAV]
                recip = outp.tile([P, 1], F32, tag="recip", name=f"rcp{i}")
                nc.vector.reciprocal(recip[:], av[:, P : P + 1])
                o_sb = outp.tile([P, H], F32, tag="osb", name=f"osb{i}")
                nc.vector.tensor_scalar_mul(o_sb[:], av[:, 0:P], recip[:])
                nc.sync.dma_start(y_d[i * P : (i + 1) * P, :], o_sb[:])

            # one-round software pipeline: round j computes scores row
            # j+1 BEFORE row j's AV updates so by the time the PE
            # reaches an AV matmul its exp input finished a full round
            # earlier and the PE never blocks waiting on ScalarE.
            scores_row(0)
            transpose_v(0)
            for ch in (1, 2):
                sl = slice(ch * CHW, (ch + 1) * CHW)
                nc.vector.tensor_copy(vt_sb[:, sl], vt_ps[ch][:])
            for j in range(NT):
                if j + 1 < NT:
                    scores_row(j + 1)
                    transpose_v(j + 1)
                if j == 0:
                    for i in range(min(WIN, NT)):
                        av_banks[i] = ps.tile(
                            [P, CHW], F32, tag="ps", name=f"avb{i}"
                        )
                else:
                    # column activated this round catches up rows 0..j-1
                    act = j + WIN - 1
                    if act < NT:
                        av_banks[act] = ps.tile(
                            [P, CHW], F32, tag="ps", name=f"avb{act}"
                        )
                hi = min(j + WIN, NT) if j == 0 else min(j + WIN - 1, NT)
                for i in range(j, hi):
                    av_update(j, i, start=(j == 0), stop=(j == i))
                # the newly activated column (rows 0..j) comes after the
                # window updates so the recycled bank's WAR on last
                # round's normalize hides behind them
                if j > 0 and j + WIN - 1 < NT:
                    act = j + WIN - 1
                    for jc in range(j + 1):
                        av_update(jc, act, start=(jc == 0), stop=False)
                normalize_out(j)

    nc.compile()
    return nc


_NC_CACHE = None


def _get_module():
    global _NC_CACHE
    if _NC_CACHE is None:
        _NC_CACHE = build_module()
    return _NC_CACHE


def run(in_maps, trace=False, **kw):
    nc = _get_module()
    return bass_utils.run_bass_kernel_spmd(
        nc, in_maps, core_ids=list(range(NCORES)), trace=trace, **kw
    )


def make_in_maps(x, W_key, W_value):
    x = np.asarray(x, dtype=np.float32).astype(np.float16)
    xT = np.ascontiguousarray(x.transpose(0, 2, 1))
    wk = np.asarray(W_key, np.float32).astype(np.float16)
    wk = np.ascontiguousarray(wk.reshape(NC, P, H).transpose(1, 0, 2))
    wv = np.asarray(W_value, np.float32).astype(np.float16)
    wv = np.ascontiguousarray(wv.reshape(NC, P, H).transpose(1, 0, 2))
    return [{"xT": xT[b], "WK": wk, "WV": wv} for b in range(NCORES)]


def kernel(x, W_key, W_query, W_value):
    # W_query intentionally unused: the reference applies W_key for q too.
    del W_query
    res = run(make_in_maps(x, W_key, W_value), trace=False)
    return np.stack([res.results[b]["y"] for b in range(NCORES)], axis=0)


# revision 16
# speedup vs baseline: 1.7175x; 1.0238x over previous
"""Trainium2 Bass kernel for a single causal attention head (with the
faithful source bug: q = x @ W_key, W_query unused).

Full-input contract: kernel(x, W_key, W_query, W_value) -> [8, 2048, 128].
Sharding: data-parallel over batch B=8 across 8 NeuronCores (1 batch/core).

Per-core math (T=2048, C=1024, H=128):
    K = x @ W_key            (V = x @ W_value)
    S = K @ K.T * H**-0.5    (symmetric since q == k)
    out = softmax(causal(S)) @ V

Schedule:
  - All input DMA triggers issue before anything else, on hardware-DGE
    queues only (sync/scalar/vector — gpsimd dma_start is software-DGE
    at ~750ns/trigger and must not carry input DMAs). wk/wv are split
    tensors so the first LDWEIGHTS gates on a 64-descriptor transfer;
    xt0 is DMA'd in two halves so the first matmul gates on 32
    descriptors. The ACT exp-table warm runs after the scalar queue's
    triggers; constants (identity/diag mask) build on gpsimd which has
    nothing else to do.
  - Projections run tile-major (K chunks then V chunks per c-tile),
    chasing the DMA. On the last c-tile the K chunks are emitted first
    with the PSUM->SBUF fp16 casts interleaved on scalar/vector, and
    the V chunks keep the PE busy while those casts drain. vt casts
    ch0/ch3 precede transpose_v(0) (vaug0 ASAP), ch1/ch2 follow it.
  - K kept transposed (KT [h, t], fp16). Score tile (j-rows, b-cols) =
    KT_j.T @ KT_b -> [keys j (part), queries b (free)] which is exactly
    the AV lhsT layout. Only the upper triangle is computed (S
    symmetric); causal mask is a post-exp multiply on diag tiles only.
  - exp without max-subtraction (scores bounded; fp16 E in range) in
    512-col chunks on ScalarE; softmax denominators ride the AV matmul
    as a ones-column appended to V (rhs = [v | 1], 129 cols).
  - AV columns accumulate in PSUM under a WIN=3 sliding window (one
    open accumulation group per 2KB bank); a late-activated column
    catches up earlier rows from e_all. One-round software pipeline:
    scores row j+1 issues before row j's AV updates so the in-order PE
    never waits on ScalarE's exp.
"""

import numpy as np

import concourse.bass as bass
import concourse.mybir as mybir
import concourse.tile as tile
from concourse import bacc, bass_utils
from concourse.masks import make_identity, make_upper_triangular


P = 128
T = 2048
C = 1024
H = 128
NT = T // P  # 16 seq tiles
NC = C // P  # 8 contraction tiles
NCORES = 8
SCALE = float(H) ** -0.5
F32 = mybir.dt.float32
FP16 = mybir.dt.float16
EXP = mybir.ActivationFunctionType.Exp
CHW = 512
CHN = T // CHW  # 4 chunks


def build_module():
    nc = bacc.Bacc(
        "TRN2", target_bir_lowering=False, debug=False, num_devices=NCORES
    )
    xT_d = nc.dram_tensor("xT", [C, T], FP16, kind="ExternalInput").ap()
    # weights pre-arranged [p, kv, c, h] on the host: one fused DMA
    # with full 4KB rows per partition
    w_d = nc.dram_tensor("W", [P, 2, NC, H], FP16, kind="ExternalInput").ap()
    y_d = nc.dram_tensor("y", [T, H], F32, kind="ExternalOutput").ap()

    # offsets of score row-block j inside e_all (block j holds queries
    # b in [j*128, 2048) -> width (NT-j)*128)
    offs = []
    off = 0
    for j in range(NT):
        offs.append(off)
        off += (NT - j) * P
    e_width = off  # 136 * 128 = 17408

    with tile.TileContext(nc) as tc:
        with (
            tc.tile_pool(name="const", bufs=1) as const,
            tc.tile_pool(name="xt", bufs=8) as xt_pool,
            tc.tile_pool(name="kv", bufs=1) as kv,
            tc.tile_pool(name="e", bufs=1) as e_pool,
            tc.tile_pool(name="outp", bufs=4) as outp,
            tc.tile_pool(name="ps", bufs=8, space="PSUM") as ps,
        ):
            # ---- input DMA triggers first, nothing ahead of them ----
            # baseline DMA granularity (one 4KB-row transfer per tile:
            # finer splits add descriptor overhead and congest the
            # rings); xt0 on scalar so it isn't queued behind w
            w_sb = const.tile([P, 2, NC, H], FP16)
            wk_sb = w_sb[:, 0]
            wv_sb = w_sb[:, 1]
            xt = [
                xt_pool.tile([P, T], FP16, tag="xt", name=f"xt{c}")
                for c in range(NC)
            ]
            nc.scalar.dma_start(xt[0][:], xT_d[0:P, :])
            nc.sync.dma_start(w_sb[:], w_d[:])
            nc.sync.dma_start(xt[1][:], xT_d[P : 2 * P, :])
            nc.scalar.dma_start(xt[2][:], xT_d[2 * P : 3 * P, :])
            nc.sync.dma_start(xt[3][:], xT_d[3 * P : 4 * P, :])
            nc.scalar.dma_start(xt[4][:], xT_d[4 * P : 5 * P, :])
            nc.sync.dma_start(xt[5][:], xT_d[5 * P : 6 * P, :])
            nc.scalar.dma_start(xt[6][:], xT_d[6 * P : 7 * P, :])
            nc.sync.dma_start(xt[7][:], xT_d[7 * P : 8 * P, :])

            # pre-warm the ACT exp table (scalar queue, after triggers)
            warm = const.tile([P, 1], F32)
            nc.vector.memset(warm[:], 0.0)
            nc.scalar.activation(warm[:], warm[:], EXP)

            # constants on gpsimd (free of triggers); fp16 casts on
            # gpsimd too so the vector queue stays clear for A-end
            ident_f = const.tile([P, P], F32)
            make_identity(nc, ident_f)
            dmask_f = const.tile([P, P], F32)
            make_upper_triangular(nc, dmask_f, val=1.0, diag=True)
            ident = const.tile([P, P], FP16)
            nc.gpsimd.tensor_copy(ident[:], ident_f[:])
            dmask = const.tile([P, P], FP16)
            nc.gpsimd.tensor_copy(dmask[:], dmask_f[:])

            kt_r = kv.tile([P, T], FP16)  # K^T [h, t]
            vt_sb = kv.tile([P, T], FP16)  # V^T [h, t]
            # per key-tile j: [v (128) | ones (1)]
            vaug = kv.tile([P, NT, P + 1], FP16)
            nc.vector.memset(vaug[:, :, P : P + 1], 1.0)
            e_all = e_pool.tile([P, e_width], FP16)

            # ---- A: projections, tile-major, chasing the DMA ----
            kt_ps = [
                ps.tile([P, CHW], F32, tag="ps", name=f"ktps{ch}")
                for ch in range(CHN)
            ]
            vt_ps = [
                ps.tile([P, CHW], F32, tag="ps", name=f"vtps{ch}")
                for ch in range(CHN)
            ]
            for c in range(NC):
                last = c == NC - 1
                for ch in range(CHN):
                    rhs = xt[c][:, ch * CHW : (ch + 1) * CHW]
                    nc.tensor.matmul(
                        kt_ps[ch][:],
                        wk_sb[:, c, :],
                        rhs,
                        start=(c == 0),
                        stop=last,
                    )
                    if last:
                        # kt casts interleave on scalar/vector while
                        # the V proj matmuls keep the PE busy
                        sl = slice(ch * CHW, (ch + 1) * CHW)
                        if ch % 2 == 0:
                            nc.scalar.copy(kt_r[:, sl], kt_ps[ch][:])
                        else:
                            nc.vector.tensor_copy(kt_r[:, sl], kt_ps[ch][:])
                for ch in range(CHN):
                    rhs = xt[c][:, ch * CHW : (ch + 1) * CHW]
                    nc.tensor.matmul(
                        vt_ps[ch][:],
                        wv_sb[:, c, :],
                        rhs,
                        start=(c == 0),
                        stop=last,
                    )
            # vt casts: ch0/ch3 first (transpose_v(0) needs vt chunk 0
            # and the vtr psum slot is recycled from vt_ps bank ring),
            # ch1/ch2 after transpose_v(0) so vaug0 isn't queued late
            for ch in (0, 3):
                sl = slice(ch * CHW, (ch + 1) * CHW)
                nc.vector.tensor_copy(vt_sb[:, sl], vt_ps[ch][:])

            # ---- B: scores row j / exp / V-transpose j / AV col ----
            NAV = P + 1  # v | ones

            def scores_row(j, fillers=()):
                """Emit scores+exp for row j; after each chunk's matmul
                pop a few filler thunks (AV updates) whose LDWEIGHTS
                then hide under the next 512-col score matmul."""
                fillers = list(fillers)
                n_fill = len(fillers)
                emitted = 0
                b0 = j * P
                width = T - b0
                n_chunks = -(-width // CHW)
                ci = 0
                pos = 0
                while pos < width:
                    w = min(CHW, width - pos)
                    s_ps = ps.tile([P, CHW], F32, tag="ps", name=f"sps{j}_{pos}")
                    nc.tensor.matmul(
                        s_ps[:, :w],
                        kt_r[:, b0 : b0 + P],
                        kt_r[:, b0 + pos : b0 + pos + w],
                        start=True,
                        stop=True,
                    )
                    nc.scalar.activation(
                        e_all[:, offs[j] + pos : offs[j] + pos + w],
                        s_ps[:, :w],
                        EXP,
                        scale=SCALE,
                    )
                    if pos == 0:
                        # causal mask only needed on the diagonal tile;
                        # gpsimd is idle and keeps the DVE free
                        nc.gpsimd.tensor_mul(
                            e_all[:, offs[j] : offs[j] + P],
                            e_all[:, offs[j] : offs[j] + P],
                            dmask[:],
                        )
                    pos += w
                    ci += 1
                    take = (n_fill * ci) // n_chunks - emitted
                    for th in fillers[emitted : emitted + take]:
                        th()
                    emitted += take

            def transpose_v(j):
                vtr = ps.tile([P, CHW], FP16, tag="ps", name=f"vtr{j}")
                nc.tensor.transpose(
                    vtr[:, :P], vt_sb[:, j * P : (j + 1) * P], ident[:]
                )
                nc.vector.tensor_copy(vaug[:, j, 0:P], vtr[:, :P])

            # AV columns accumulate in PSUM, at most one open
            # accumulation group per bank. A sliding window of WIN
            # concurrent columns: column i activates at round
            # max(0, i - WIN + 1), catches up rows 0..r-1 from e_all,
            # then takes one update per subsequent round.
            WIN = 3
            av_banks = {}

            def av_update(j, i, start, stop):
                eji = e_all[
                    :, offs[j] + (i - j) * P : offs[j] + (i - j + 1) * P
                ]
                nc.tensor.matmul(
                    av_banks[i][:, :NAV],
                    eji,
                    vaug[:, j, :],
                    start=start,
                    stop=stop,
                )

            def normalize_out(i):
                av = av_banks[i][:, :NAV]
                recip = outp.tile([P, 1], F32, tag="recip", name=f"rcp{i}")
                nc.vector.reciprocal(recip[:], av[:, P : P + 1])
                o_sb = outp.tile([P, H], F32, tag="osb", name=f"osb{i}")
                nc.vector.tensor_scalar_mul(o_sb[:], av[:, 0:P], recip[:])
                nc.sync.dma_start(y_d[i * P : (i + 1) * P, :], o_sb[:])

            # one-round software pipeline: round j computes scores row
            # j+1 BEFORE row j's AV updates so by the time the PE
            # reaches an AV matmul its exp input finished a full round
            # earlier and the PE never blocks waiting on ScalarE. The
            # AV updates ride as fillers between score-chunk matmuls
            # so their LDWEIGHTS hide under the 512-col streams.
            scores_row(0)
            transpose_v(0)
            for ch in (1, 2):
                sl = slice(ch * CHW, (ch + 1) * CHW)
                nc.vector.tensor_copy(vt_sb[:, sl], vt_ps[ch][:])
            for j in range(NT):
                avs = []
                if j == 0:
                    for i in range(min(WIN, NT)):
                        av_banks[i] = ps.tile(
                            [P, CHW], F32, tag="ps", name=f"avb{i}"
                        )
                    for i in range(min(WIN, NT)):
                        avs.append(
                            lambda i=i: av_update(
                                0, i, start=True, stop=(i == 0)
                            )
                        )
                else:
                    # window updates first, then the newly activated
                    # column catches up rows 0..j (its recycled bank's
                    # WAR on last round's normalize hides behind them)
                    act = j + WIN - 1
                    hi = min(j + WIN - 1, NT)
                    for i in range(j, hi):
                        avs.append(
                            lambda i=i, j=j: av_update(
                                j, i, start=False, stop=(j == i)
                            )
                        )
                    if act < NT:
                        av_banks[act] = ps.tile(
                            [P, CHW], F32, tag="ps", name=f"avb{act}"
                        )
                        for jc in range(j + 1):
                            avs.append(
                                lambda jc=jc, act=act: av_update(
                                    jc, act, start=(jc == 0), stop=False
                                )
                            )
                if j + 1 < NT:
                    scores_row(j + 1, fillers=avs)
                    transpose_v(j + 1)
                else:
                    for th in avs:
                        th()
                normalize_out(j)

    nc.compile()
    return nc


_NC_CACHE = None


def _get_module():
    global _NC_CACHE
    if _NC_CACHE is None:
        _NC_CACHE = build_module()
    return _NC_CACHE


def run(in_maps, trace=False, **kw):
    nc = _get_module()
    return bass_utils.run_bass_kernel_spmd(
        nc, in_maps, core_ids=list(range(NCORES)), trace=trace, **kw
    )


def make_in_maps(x, W_key, W_value):
    x = np.asarray(x, dtype=np.float32).astype(np.float16)
    xT = np.ascontiguousarray(x.transpose(0, 2, 1))
    wk = np.asarray(W_key, np.float32).astype(np.float16)
    wk = wk.reshape(NC, P, H).transpose(1, 0, 2)
    wv = np.asarray(W_value, np.float32).astype(np.float16)
    wv = wv.reshape(NC, P, H).transpose(1, 0, 2)
    w = np.ascontiguousarray(np.stack([wk, wv], axis=1))  # [P, 2, NC, H]
    return [{"xT": xT[b], "W": w} for b in range(NCORES)]


def kernel(x, W_key, W_query, W_value):
    # W_query intentionally unused: the reference applies W_key for q too.
    del W_query
    res = run(make_in_maps(x, W_key, W_value), trace=False)
    return np.stack([res.results[b]["y"] for b in range(NCORES)], axis=0)
